# revision 1
# baseline (speedup 1.0000x reference)
"""Sweep-variant Trainium2 kernel for nn_AttentionRNN_79078937853994.

The reference reduces to an LSTM over W=32 steps (see kernel.py docstring).
Instead of a 32-step serial loop, run K Jacobi fixed-point sweeps over the
whole sequence (measured contraction ~0.1/sweep; K=4 -> ~6e-4 abs error):

    gates^(k) = Gx + Wh^T @ H^(k-1)     4+4 matmuls into a FRESH psum tile
    gates_sb  = gh_psum + gx_sb         2 fused DVE adds (SBUF result)
    A         = sigmoid(gates_sb)       2 big ACT ops (g pre-scaled by 2)
    u         = 2*(si*sg) - si          2 DVE ops
    c         = scan(sf, u)             ONE tensor_tensor_scan (cell state!)
    h         = so * tanh(c)            1 ACT + 1 DVE (skipped last sweep)

Layouts: partitions = (batch-half, h) = 128; free = (b_loc, t) b-major, so
the scan chains along t within each batch row; segment boundaries are reset
by forcing the f-gate preactivation to -60 at t=0 columns (sigma ~ 0).
H is carried in bf16 (error floor ~2e-4) in a [128, 8, 33] buffer whose
leading column per segment is zero, giving the t-1 shift for free.

Every instruction is kept to at most ONE semaphore wait (hardware limit):
- big DMAs go through the single-queue SWDGE path,
- absorber matmuls pre-observe each DMA/memset semaphore on the PE,
- the recurrent matmuls write fresh per-sweep PSUM tensors (no long-lived
  accumulated PSUM tensor is ever read by ACT -> no forced bank chains),
- H buffers ping-pong so the h-writer never WARs the same sweep's matmuls.
"""

import json
import os
import numpy as np

import concourse.bass as bass
import concourse.mybir as mybir
import concourse.tile as tile
from concourse.bass_utils import run_bass_kernel_spmd


def _legalize_bir_waits(bir_json: bytes) -> bytes:
    """This toolchain's walrus accepts at most ONE sync wait per
    instruction.  Tile's kernel-tail drain carries one wait per live
    engine/DMA lane.  Split any excess waits onto inserted same-engine
    Drain instructions (pipeline already empty there, so they are free)."""
    d = json.loads(bir_json)
    changed = False
    for fn in d.get("functions", []):
        for bb in fn.get("blocks", []):
            insts = bb.get("instructions", [])
            out = []
            for ins in insts:
                sy = ins.get("sync_info") or {}
                ow = sy.get("on_wait") or []
                if len(ow) > 1:
                    changed = True
                    for k, w in enumerate(ow[:-1]):
                        out.append({
                            "name": f"{ins['name']}-lw{k}",
                            "opcode": "Drain",
                            "engine": ins.get("engine", "SP"),
                            "ins": [],
                            "outs": [],
                            "debug": ins.get("debug"),
                            "sync_info": {"on_wait": [w], "on_update": []},
                        })
                    sy["on_wait"] = [ow[-1]]
                out.append(ins)
            bb["instructions"] = out
    if not changed:
        return bir_json
    return json.dumps(d).encode()


def _install_bir_legalizer():
    import concourse.bass_utils as bu
    import concourse.bass2jax as b2j
    if getattr(bu, "_wait_legalizer_installed", False):
        return
    if os.environ.get("KERNEL_LDWOPT", "0") == "1":
        orig_args = bu.get_walrus_args

        def patched_args(arch, tmpdir, *, dve_root=None):
            return [a.replace("--enable-ldw-opt=false", "--enable-ldw-opt=true")
                    for a in orig_args(arch, tmpdir, dve_root=dve_root)]

        bu.get_walrus_args = patched_args
    orig = bu.compile_bir_kernel

    def patched(bir_json, tmpdir, neff_name="file.neff"):
        if isinstance(bir_json, str):
            bir_json = bir_json.encode()
        return orig(_legalize_bir_waits(bir_json), tmpdir, neff_name)

    bu.compile_bir_kernel = patched
    b2j.compile_bir_kernel = patched
    bu._wait_legalizer_installed = True


_install_bir_legalizer()

B, F, W, H = 128, 1024, 32, 64
NCORES = 8
BL = B // NCORES           # 16 batch rows per core
HB = BL // 2               # 8 rows per partition-half
G4 = 4 * H
NSWEEP = int(os.environ.get("KERNEL_NSWEEP", "3"))
FP32 = mybir.dt.float32
FP32R = mybir.dt.float32r
BF16 = mybir.dt.bfloat16
AF = mybir.ActivationFunctionType
OP = mybir.AluOpType


def build_program():
    nc = bass.Bass()

    xs = nc.declare_dram_parameter("xs", [8, 128, BL, W], FP32, isOutput=False)
    wx = nc.declare_dram_parameter("wx", [128, 8, G4], FP32, isOutput=False)
    whb = nc.declare_dram_parameter("whb", [128, G4], BF16, isOutput=False)
    bl_p = nc.declare_dram_parameter("bl", [1, G4], FP32, isOutput=False)
    ones_d = nc.declare_dram_parameter("ones", [1, BL * W], FP32, isOutput=False)
    out = nc.declare_dram_parameter("out", [BL, W, H], FP32, isOutput=True)

    C = HB * W  # 256 free columns: (b_loc, t), t innermost

    with tile.TileContext(nc) as tc:
        with (
            tc.tile_pool(name="const", bufs=1) as const,
            tc.tile_pool(name="xp", bufs=8) as xp,
            tc.tile_pool(name="gxp", bufs=1, space="PSUM") as gxp,
            tc.tile_pool(name="ghp", bufs=1, space="PSUM") as ghp,
            tc.tile_pool(name="dpsum", bufs=1, space="PSUM") as dpsum,
            tc.tile_pool(name="sweep", bufs=NSWEEP + 1) as swp,
            tc.tile_pool(name="hbuf", bufs=1) as hbufp,
            tc.tile_pool(name="osb", bufs=1) as osb,
        ):
            wx_sb = const.tile([128, 8, G4], FP32R)
            wh_sb = const.tile([128, G4], BF16)   # Wh stacked for both halves
            b_sb = const.tile([1, G4], FP32R)
            ones_sb = const.tile([1, BL * W], FP32R)
            warm_sb = const.tile([1, 4], FP32)
            gx_sb = const.tile([128, 4, C], FP32)

            # H ping-pong buffers, bf16, leading zero column per b segment.
            hA = hbufp.tile([128, HB, W + 2], BF16, tag="hA")
            hB = hbufp.tile([128, HB, W + 2], BF16, tag="hB")
            nc.gpsimd.memset(hA[:].bitcast(FP32), 0.0)
            nc.gpsimd.memset(hB[:].bitcast(FP32), 0.0)

            # Trigger order = consumption order: wx (the PE absorber's
            # gate), then xs, then the late-needed small tensors.
            nc.sync.dma_start(wx_sb[:], wx[:].bitcast(FP32R))
            xtiles = []
            for j in range(8):
                xj = xp.tile([128, BL, W], FP32R, name=f"xj{j}")
                nc.sync.dma_start(xj[:], xs[j].bitcast(FP32R))
                xtiles.append(xj)
            nc.sync.dma_start(wh_sb[:], whb[:])
            nc.sync.dma_start(b_sb[:], bl_p[:].bitcast(FP32R))
            nc.sync.dma_start(ones_sb[:], ones_d[:].bitcast(FP32R))
            nc.gpsimd.memset(warm_sb[:], 0.5)

            # ACT table warmup (sigmoid set includes tanh) during the DMAs.
            nc.scalar.activation(warm_sb[0:1, 0:2], warm_sb[0:1, 0:2], AF.Sigmoid)
            nc.scalar.activation(warm_sb[0:1, 2:4], warm_sb[0:1, 0:2], AF.Tanh)

            # One-wait absorber (matmuls may carry at most one sync wait).
            dp = dpsum.tile([128, 256], FP32)
            nc.tensor.matmul(dp[0:H, :], wx_sb[:, 0, 0:H], wx_sb[:, 0, :])

            # ---- Phase 1: Gx + b -> PSUM, both halves at base-0 ------------
            # (this walrus rejects fp32r matmuls with output base != 0, so
            # half 1 is computed at base 0 and moved up with an SBUF->SBUF
            # DMA, the only partition-crossing path outside the PE)
            # Gate-PAIRED matmuls: lhsT = Wx[:, (i,f)] or (g,o) -> M=128,
            # N=512; 18 matmuls instead of 36 (LDWEIGHTS is not pipelined
            # in this walrus, so matmul count dominates phase 1).
            # Output partitions are (gate-of-pair, h); columns are (b, t).
            p_if = gxp.tile([128, BL * W], FP32, tag="pif")
            p_go = gxp.tile([128, BL * W], FP32, tag="pgo")
            for j in range(8):
                for pr, ps_t in ((0, p_if), (1, p_go)):
                    nc.tensor.matmul(
                        ps_t[:],
                        wx_sb[:, j, bass.ts(pr, 128)],
                        xtiles[j][:],
                        start=(j == 0), stop=False,
                        skip_group_check=True,
                    )
            # bias last (accumulation is commutative); absorbers first so
            # each matmul needs a single wait
            nc.tensor.matmul(dp[0:H, :], wh_sb[0:H, 0:H], wh_sb[0:H, :])
            nc.tensor.matmul(dp[0:H, :], b_sb[0:1, 0:H], b_sb[0:1, :])
            nc.tensor.matmul(dp[0:H, 0:128], ones_sb[0:1, 0:H], ones_sb[0:1, 0:128])
            for pr, ps_t in ((0, p_if), (1, p_go)):
                nc.tensor.matmul(
                    ps_t[:], b_sb[0:1, bass.ts(pr, 128)], ones_sb[0:1, :],
                    start=False, stop=True, skip_group_check=True,
                )

            # Assemble gx_sb [128=(hf,h), 4, C].  Partition-aligned pieces go
            # by DVE copy; the four partition-crossing pieces stage through
            # SBUF and move with two SBUF->SBUF DMAs (XOR-64 partition swap).
            gsv = gx_sb[:].rearrange("p (u v) c -> p v u c", v=2)
            st = const.tile([128, 2, C], FP32)
            nc.vector.tensor_copy(st[0:H, 0, :], p_if[0:H, C:])       # i hf1
            nc.vector.tensor_copy(st[0:H, 1, :], p_go[0:H, C:])       # g hf1
            nc.vector.tensor_copy(st[H:128, 0, :], p_if[H:128, 0:C])  # f hf0
            nc.vector.tensor_copy(st[H:128, 1, :], p_go[H:128, 0:C])  # o hf0
            nc.gpsimd.dma_start(gsv[H:128, 0], st[0:H, :, :])
            nc.gpsimd.dma_start(gsv[0:H, 1], st[H:128, :, :])
            nc.vector.tensor_copy(gx_sb[0:H, 0, :], p_if[0:H, 0:C])     # i hf0
            nc.vector.tensor_copy(gx_sb[0:H, 2, :], p_go[0:H, 0:C])     # g hf0
            nc.vector.tensor_copy(gx_sb[H:128, 1, :], p_if[H:128, C:])  # f hf1
            nc.vector.tensor_copy(gx_sb[H:128, 3, :], p_go[H:128, C:])  # o hf1
            # force sigma(f) ~ 0 at segment starts (scan boundary reset)
            gx_f = gx_sb[:, 1, :].rearrange("p (b t) -> p b t", t=W)
            nc.vector.memset(gx_f[:, :, 0:1], -60.0)

            # ---- Phase 2: K fixed-point sweeps -----------------------------
            # One persistent gh tensor; each sweep's matmuls rewrite it with
            # start=True.  After the adds, 1-element DVE memsets make DVE the
            # banks' last writer so the next sweep's matmuls carry only a
            # single (DVE) wait.
            gh = ghp.tile([128, 4, C], FP32)
            c_all = None
            for k in range(NSWEEP):
                hw_cur, hw_prev = (hA, hB) if k % 2 == 0 else (hB, hA)
                if k == 0:
                    gates = gx_sb
                else:
                    for g in (1, 3, 0, 2):            # f, o first
                        for hf in range(2):
                            nc.tensor.matmul(
                                gh[bass.ts(hf, H), g, :],
                                wh_sb[bass.ts(hf, H), bass.ts(g, H)],
                                hw_prev[bass.ts(hf, H), :, 0:W],
                                start=True, stop=True, skip_group_check=True,
                            )
                    gates = swp.tile([128, 4, C], FP32, tag="gates")
                    gav = gates[:].rearrange("p (u v) c -> p v u c", v=2)
                    ghv = gh[:].rearrange("p (u v) c -> p v u c", v=2)
                    nc.vector.tensor_tensor(gav[:, 1], ghv[:, 1], gsv[:, 1], OP.add)
                    nc.vector.tensor_tensor(gav[:, 0], ghv[:, 0], gsv[:, 0], OP.add)
                    nc.vector.memset(gh[0:1, 0, 0:1], 0.0)
                    nc.vector.memset(gh[0:1, 2, 0:1], 0.0)

                a = swp.tile([128, 4, C], FP32, tag="a")
                av = a[:].rearrange("p (u v) c -> p v u c", v=2)
                gv = gates[:].rearrange("p (u v) c -> p v u c", v=2)
                nc.scalar.activation(av[:, 1], gv[:, 1], AF.Sigmoid)  # f, o
                nc.scalar.activation(av[:, 0], gv[:, 0], AF.Sigmoid)  # i, g

                si, sf, sg, so = a[:, 0, :], a[:, 1, :], a[:, 2, :], a[:, 3, :]
                m = swp.tile([128, C], FP32, tag="m")
                nc.vector.tensor_tensor(m[:], si, sg, OP.mult)
                u = swp.tile([128, C], FP32, tag="u")
                nc.vector.scalar_tensor_tensor(u[:], m[:], 2.0, si,
                                               OP.mult, OP.subtract)
                c_all = swp.tile([128, C], FP32, tag="c")
                nc.vector.tensor_tensor_scan(c_all[:], sf, u[:], 0.0,
                                             OP.mult, OP.add)
                if k < NSWEEP - 1:
                    tcs = swp.tile([128, C], FP32, tag="tc")
                    nc.scalar.activation(tcs[:], c_all[:], AF.Tanh)
                    so3 = so.rearrange("p (b t) -> p b t", t=W)
                    tc3 = tcs[:].rearrange("p (b t) -> p b t", t=W)
                    nc.vector.tensor_tensor(hw_cur[:, :, 1:W + 1], so3, tc3,
                                            OP.mult)

            # ---- Phase 3: DVE 32x32 block-transpose + strided stores ----
            # c_all[p=(hf,h), c=(b_loc,t)]: t is the inner-32 of the free
            # dim and h%32 the inner-32 of partitions, so a 32x32 block
            # transpose yields bt[32*(p//32)+t, 32*b_loc+h%32].
            bt = swp.tile([128, C], FP32, tag="bt")
            nc.vector.transpose(bt[:], c_all[:])
            # Absorber: Pool observes the DVE semaphore here so each output
            # DMA below carries only its single lane-reuse wait.
            pool_scratch = swp.tile([1, 2], FP32, tag="ps")
            nc.gpsimd.tensor_copy(pool_scratch[:], bt[0:1, 0:2])
            btv = bt[:].rearrange("(q t) c -> q t c", q=4)
            out_v = out.rearrange("(hf bl) t (hi hm) -> hf hi t bl hm",
                                  hf=2, hi=2)
            for hf in range(2):
                for hi in range(2):
                    nc.sync.dma_start(out_v[hf, hi], btv[2 * hf + hi])

    return nc


_CACHE = {}


def _get_program():
    if "nc" not in _CACHE:
        _CACHE["nc"] = build_program()
    return _CACHE["nc"]


def _to_bf16(a):
    import ml_dtypes
    return np.ascontiguousarray(a.astype(ml_dtypes.bfloat16))


def make_in_maps(x, Wx, Wh, b_lstm):
    x = np.ascontiguousarray(np.asarray(x, np.float32))
    Wx = np.asarray(Wx, np.float32).copy()
    Wh = np.asarray(Wh, np.float32).copy()
    b = np.asarray(b_lstm, np.float32).copy()
    Wx[:, 2 * H:3 * H] *= 2.0
    Wh[:, 2 * H:3 * H] *= 2.0
    b[2 * H:3 * H] *= 2.0

    wx_p = np.ascontiguousarray(Wx.reshape(128, 8, G4))
    wh_bf = _to_bf16(np.vstack([Wh, Wh]))                 # [128, 4H]
    b_p = np.ascontiguousarray(b.reshape(1, G4))
    ones_h = np.ones((1, BL * W), np.float32)

    in_maps = []
    for core in range(NCORES):
        shard = x[core * BL:(core + 1) * BL]              # [16, 1024, 32]
        # xs[j, p, b, t] = shard[b, 8p + j, t]
        xsp = shard.reshape(BL, 128, 8, W).transpose(2, 1, 0, 3)
        in_maps.append({
            "xs": np.ascontiguousarray(xsp),
            "wx": wx_p,
            "whb": wh_bf,
            "bl": b_p,
            "ones": ones_h,
        })
    return in_maps


def kernel(x, W_state, b_state, W_in, w_attn, b_attn, Wx, Wh, b_lstm):
    nc = _get_program()
    in_maps = make_in_maps(x, Wx, Wh, b_lstm)
    trace = bool(int(os.environ.get("KERNEL_TRACE", "0")))
    res = run_bass_kernel_spmd(
        nc, in_maps, core_ids=list(range(NCORES)),
        trace=trace, trace_cores=list(range(NCORES)) if trace else None,
    )
    _CACHE["last_result"] = res
    outp = np.empty((B, W, H), np.float32)
    for core in range(NCORES):
        outp[core * BL:(core + 1) * BL] = res.results[core]["out"]
    return outp



# revision 15
# speedup vs baseline: 1.4821x; 1.4821x over previous
"""Trainium2 kernel for nn_AttentionRNN_79078937853994 (v2).

The reference reduces to an LSTM over W=32 steps (attention is dead code:
softmax over a size-1 axis == 1).  K Jacobi fixed-point sweeps replace the
serial loop (contraction ~0.1/sweep; K=2 -> ~8.5e-3 rel err, K=3 -> ~9e-4).

v2 layout: partitions = (batch-half hf, h) = 128; free = (slot, b_loc, t)
with slots (i, g, f, o).  Phase 1 computes Gx directly in this layout with
col-tiled bf16 matmuls (tile_position inferred from base partitions): for
each f-chunk j and slot s, two concurrent matmuls (one per batch half) of
N=256.  Gates live in PSUM for the whole kernel; sweep k's recurrent
matmuls ACCUMULATE Wh^T @ (h_k - h_{k-1}) on top (start=False), so there
are no DVE gate adds and no SBUF gx assembly at all.  Bias and the f-gate
t=0 reset (-60, scan segment boundary) are rank-1 matmuls.

Everything is bf16 except PSUM and the final scan output (fp32); the scan
keeps fp32 state internally so bf16 operands do not compound error.
Dummy matmuls on a junk tile warm the PE HAM clock gate during the input
DMA window.  Output is ONE linear [128, 256] DMA; the host unscrambles.
"""

import json
import os
import numpy as np

import concourse.bass as bass
import concourse.mybir as mybir
import concourse.tile as tile
from concourse.bass_utils import run_bass_kernel_spmd


def _legalize_bir_waits(bir_json: bytes) -> bytes:
    """This toolchain's walrus accepts at most ONE sync wait per
    instruction.  Tile's kernel-tail drain carries one wait per live
    engine/DMA lane.  Split any excess waits onto inserted same-engine
    Drain instructions (pipeline already empty there, so they are free)."""
    d = json.loads(bir_json)
    changed = False
    for fn in d.get("functions", []):
        for bb in fn.get("blocks", []):
            insts = bb.get("instructions", [])
            out = []
            for ins in insts:
                sy = ins.get("sync_info") or {}
                ow = sy.get("on_wait") or []
                if len(ow) > 1:
                    changed = True
                    for k, w in enumerate(ow[:-1]):
                        out.append({
                            "name": f"{ins['name']}-lw{k}",
                            "opcode": "Drain",
                            "engine": ins.get("engine", "SP"),
                            "ins": [],
                            "outs": [],
                            "debug": ins.get("debug"),
                            "sync_info": {"on_wait": [w], "on_update": []},
                        })
                    sy["on_wait"] = [ow[-1]]
                out.append(ins)
            bb["instructions"] = out
    if not changed:
        return bir_json
    return json.dumps(d).encode()


def _install_bir_legalizer():
    import concourse.bass_utils as bu
    import concourse.bass2jax as b2j
    if getattr(bu, "_wait_legalizer_installed", False):
        return
    if os.environ.get("KERNEL_LDWOPT", "0") == "1":
        orig_args = bu.get_walrus_args

        def patched_args(arch, tmpdir, *, dve_root=None):
            return [a.replace("--enable-ldw-opt=false", "--enable-ldw-opt=true")
                    for a in orig_args(arch, tmpdir, dve_root=dve_root)]

        bu.get_walrus_args = patched_args
    orig = bu.compile_bir_kernel

    def patched(bir_json, tmpdir, neff_name="file.neff"):
        if isinstance(bir_json, str):
            bir_json = bir_json.encode()
        return orig(_legalize_bir_waits(bir_json), tmpdir, neff_name)

    bu.compile_bir_kernel = patched
    b2j.compile_bir_kernel = patched
    bu._wait_legalizer_installed = True


_install_bir_legalizer()

B, F, W, H = 128, 1024, 32, 64
NCORES = 8
BL = B // NCORES           # 16 batch rows per core
HB = BL // 2               # 8 rows per partition-half
C = HB * W                 # 256 free columns per half: (b_loc, t), t inner
WP = W + 2                 # h buffers padded: col 0 unused, col 1 = zero
NSWEEP = int(os.environ.get("KERNEL_NSWEEP", "2"))
NWARM = int(os.environ.get("KERNEL_NWARM", "0"))
DEBUG_GX = os.environ.get("KERNEL_DEBUG_GX", "0") == "1"
FP32 = mybir.dt.float32
BF16 = mybir.dt.bfloat16
AF = mybir.ActivationFunctionType
OP = mybir.AluOpType


def build_program():
    nc = bass.Bass()

    # x packed two f-chunks per DMA: 2 KiB per partition line keeps the
    # SDMA descriptors at full rate (1 KiB lines measured at half rate)
    xs = nc.declare_dram_parameter("xs", [4, 128, 2, BL, W], BF16,
                                   isOutput=False)
    wx = nc.declare_dram_parameter("wx", [128, 8, 4, H], BF16, isOutput=False)
    wh = nc.declare_dram_parameter("wh", [128, 4, H], BF16, isOutput=False)
    msc = nc.declare_dram_parameter("msc", [1, 1024], BF16, isOutput=False)
    out = nc.declare_dram_parameter("out", [128, C], FP32, isOutput=True)
    dbg = (nc.declare_dram_parameter("dbg", [128, 4 * C], FP32, isOutput=True)
           if DEBUG_GX else None)

    with tile.TileContext(nc) as tc:
        with (
            tc.tile_pool(name="gatesp", bufs=1, space="PSUM") as gatesp,
            tc.tile_pool(name="warmp", bufs=1, space="PSUM") as warmp,
            tc.tile_pool(name="const", bufs=1) as const,
        ):
            gates = gatesp.tile([128, 4, C], FP32)     # banks 0-1, resident
            wps = warmp.tile([128, 512], FP32)         # warm-up garbage bank

            wx_sb = const.tile([128, 8, 4, H], BF16)
            wh_sb = const.tile([128, 4, H], BF16)
            msc_sb = const.tile([1, 1024], BF16)
            junk = const.tile([128, 512], BF16)
            warm_sb = const.tile([1, 4], FP32)
            xt2 = [const.tile([128, 2, BL, W], BF16, name=f"xt{j}")
                   for j in range(4)]
            hbufs = [const.tile([128, HB, WP], BF16, name=f"hb{k}")
                     for k in range(max(NSWEEP - 1, 1))]
            dlt = (const.tile([128, HB, WP], BF16, name="dlt")
                   if NSWEEP > 2 else None)

            # zero-init h buffers (only col 1 must be zero, but a full
            # memset is cheap and runs during the DMA window) + junk tile
            nc.gpsimd.memset(junk[:].bitcast(FP32), 0.0)
            for hb in hbufs:
                nc.gpsimd.memset(hb[:].bitcast(FP32), 0.0)
            if dlt is not None:
                nc.gpsimd.memset(dlt[:].bitcast(FP32), 0.0)
            nc.gpsimd.memset(warm_sb[:], 0.5)

            # Trigger order = consumption order: wx gates the first real
            # matmul; msc next so the zero/bias matmuls run during the x
            # DMAs; wh is needed only at sweep 1.
            nc.sync.dma_start(wx_sb[:], wx[:])
            nc.sync.dma_start(msc_sb[:], msc[:])
            for jj in range(4):
                nc.sync.dma_start(xt2[jj][:], xs[jj])
            nc.sync.dma_start(wh_sb[:], wh[:])

            # ACT table warm-up (sigmoid set includes tanh) during DMAs.
            nc.scalar.activation(warm_sb[0:1, 0:2], warm_sb[0:1, 0:2],
                                 AF.Sigmoid)
            nc.scalar.activation(warm_sb[0:1, 2:4], warm_sb[0:1, 0:2],
                                 AF.Tanh)

            # (optional) PE HAM warm-up — measured ineffective on these
            # parts (PE stays ~1 GHz), so NWARM defaults to 0.
            for k in range(NWARM):
                nc.tensor.matmul(wps[:], junk[:, 0:128], junk[:],
                                 start=True, stop=True,
                                 skip_group_check=True)

            # ---- Phase 1: Gx -> PSUM, direct (hf, h) layout -------------
            # One K=1 zero-matmul per bank claims the whole bank first
            # (start=True marks the full 2 KiB zero-region; writing every
            # byte clears it and sets has_written everywhere).  Every later
            # matmul uses start=False and is therefore ORDER-INDEPENDENT —
            # Tile may reorder them freely without corrupting accumulation.
            # The WAW overlap with the zero-matmul keeps them ordered after
            # it.  The zero/bias/reset matmuls depend only on junk/msc, so
            # they execute during the x DMA wait — off the critical path.
            for half in range(2):
                nc.tensor.matmul(
                    gates[:, 2 * half:2 * half + 2, :],
                    junk[0:1, 0:128], junk[0:1, 0:512],
                    start=True, stop=False, skip_group_check=True,
                )
            # bias (rank-1, misc[512+128s:...] = [b_s | b_s]) and the
            # f-gate t=0 reset: ones x (-60 pattern) into slot 2.
            for s in range(4):
                nc.tensor.matmul(
                    gates[:, s, :],
                    msc_sb[0:1, 512 + 128 * s:512 + 128 * (s + 1)],
                    msc_sb[0:1, 0:C],
                    start=False, stop=False, skip_group_check=True,
                )
            nc.tensor.matmul(
                gates[:, 2, :], msc_sb[0:1, 0:128], msc_sb[0:1, C:2 * C],
                start=False, stop=False, skip_group_check=True,
            )
            # Per f-chunk j and slot s: two col-tiled matmuls (batch halves
            # run concurrently in the PE array; tile_position inferred from
            # output base partition).
            for j in range(8):
                for s in range(4):
                    for hf in range(2):
                        nc.tensor.matmul(
                            gates[bass.ts(hf, H), s, :],
                            wx_sb[:, j, s, :],
                            xt2[j // 2][:, j % 2, bass.ts(hf, HB), :],
                            start=False,
                            stop=(j == 7 and hf == 1 and s in (1, 3)),
                            skip_group_check=True,
                        )

            if DEBUG_GX:
                dbg_sb = const.tile([128, 4, C], FP32, name="dbg_sb")
                nc.vector.tensor_copy(dbg_sb[:, 0:2, :], gates[:, 0:2, :])
                nc.vector.tensor_copy(dbg_sb[:, 2:4, :], gates[:, 2:4, :])
                nc.sync.dma_start(dbg[:], dbg_sb[:].rearrange(
                    "p s c -> p (s c)"))

            # ---- Phase 2: K fixed-point sweeps --------------------------
            c_out = const.tile([128, C], FP32, name="c_out")
            for k in range(NSWEEP):
                last = (k == NSWEEP - 1)
                if k > 0:
                    # gates += Wh^T @ delta_h  (delta = h_0 on sweep 1)
                    src = hbufs[0] if k == 1 else dlt
                    for s in range(4):
                        if last and s == 3:
                            continue     # o-gate unused on the last sweep
                        for hf in range(2):
                            nc.tensor.matmul(
                                gates[bass.ts(hf, H), s, :],
                                wh_sb[bass.ts(hf, H), s, :],
                                src[bass.ts(hf, H), :, 1:W + 1],
                                start=False, stop=True,
                                skip_group_check=True,
                            )

                s_ig = const.tile([128, 2, C], BF16, name=f"sig{k}")
                nc.scalar.activation(s_ig[:], gates[:, 0:2, :], AF.Sigmoid)
                if last:
                    s_f = const.tile([128, C], BF16, name=f"sf{k}")
                    nc.scalar.activation(s_f[:], gates[:, 2, :], AF.Sigmoid)
                    sf, so = s_f[:], None
                else:
                    s_fo = const.tile([128, 2, C], BF16, name=f"sfo{k}")
                    nc.scalar.activation(s_fo[:], gates[:, 2:4, :],
                                         AF.Sigmoid)
                    sf, so = s_fo[:, 0, :], s_fo[:, 1, :]

                si, sg = s_ig[:, 0, :], s_ig[:, 1, :]
                # u = si * tanh(g_pre) with tanh(g) = 2*sigmoid(2g) - 1:
                # tensor_scalar gets 4x bf16 mode, tensor_tensor 2x
                # (scalar_tensor_tensor measured stuck at 1x).
                v = const.tile([128, C], BF16, name=f"v{k}")
                nc.vector.tensor_scalar(v[:], sg, 2.0, -1.0,
                                        OP.mult, OP.add)
                u = const.tile([128, C], BF16, name=f"u{k}")
                nc.vector.tensor_tensor(u[:], si, v[:], OP.mult)
                if last:
                    nc.vector.tensor_tensor_scan(c_out[:], sf, u[:], 0.0,
                                                 OP.mult, OP.add)
                else:
                    ck = const.tile([128, C], BF16, name=f"c{k}")
                    nc.vector.tensor_tensor_scan(ck[:], sf, u[:], 0.0,
                                                 OP.mult, OP.add)
                    tck = const.tile([128, C], BF16, name=f"tc{k}")
                    nc.scalar.activation(tck[:], ck[:], AF.Tanh)
                    h_cur = hbufs[k]
                    so3 = so.rearrange("p (b t) -> p b t", t=W)
                    tc3 = tck[:].rearrange("p (b t) -> p b t", t=W)
                    nc.vector.tensor_tensor(h_cur[:, :, 2:W + 2], so3, tc3,
                                            OP.mult)
                    if k > 0 and not last:
                        nc.vector.tensor_tensor(
                            dlt[:, :, 2:W + 2], h_cur[:, :, 2:W + 2],
                            hbufs[k - 1][:, :, 2:W + 2], OP.subtract)

            nc.sync.dma_start(out[:], c_out[:])

    return nc


_CACHE = {}


def _get_program():
    if "nc" not in _CACHE:
        _CACHE["nc"] = build_program()
    return _CACHE["nc"]


def _to_bf16(a):
    import ml_dtypes
    return np.ascontiguousarray(np.asarray(a, np.float32).astype(
        ml_dtypes.bfloat16))


# keras gate order in the 4H axis is (i, f, g, o); our slot order (i,g,f,o)
_SLOT = [0, 2, 1, 3]


def make_in_maps(x, Wx, Wh, b_lstm):
    x = np.asarray(x, np.float32)
    Wxs = np.asarray(Wx, np.float32).copy()
    Whs = np.asarray(Wh, np.float32).copy()
    bs = np.asarray(b_lstm, np.float32).copy()
    Wxs[:, 2 * H:3 * H] *= 2.0       # tanh(g) = 2*sigmoid(2g) - 1
    Whs[:, 2 * H:3 * H] *= 2.0
    bs[2 * H:3 * H] *= 2.0

    wx_p = _to_bf16(Wxs.reshape(F, 4, H)[:, _SLOT, :]
                    .reshape(128, 8, 4, H))
    wh4 = Whs.reshape(H, 4, H)[:, _SLOT, :]
    wh_p = _to_bf16(np.concatenate([wh4, wh4], axis=0))
    b4 = bs.reshape(4, H)[_SLOT]
    msc = np.zeros((1, 1024), np.float32)
    msc[0, 0:C] = 1.0                          # ones (bias rhs / reset lhsT)
    msc[0, C + np.arange(HB) * W] = -60.0      # t=0 reset pattern
    for s in range(4):
        msc[0, 512 + 128 * s:512 + 128 * s + H] = b4[s]
        msc[0, 512 + 128 * s + H:512 + 128 * (s + 1)] = b4[s]
    msc_p = _to_bf16(msc)

    xb = _to_bf16(x)
    in_maps = []
    for core in range(NCORES):
        shard = xb[core * BL:(core + 1) * BL]      # [16, 1024, 32]
        # xs[jj, p, i, b, t] = shard[b, 8p + 2*jj + i, t]  (2 chunks/DMA)
        xsp = np.ascontiguousarray(
            shard.reshape(BL, 128, 4, 2, W).transpose(2, 1, 3, 0, 4))
        in_maps.append({
            "xs": xsp, "wx": wx_p, "wh": wh_p, "msc": msc_p,
        })
    return in_maps


def kernel(x, W_state, b_state, W_in, w_attn, b_attn, Wx, Wh, b_lstm):
    nc = _get_program()
    in_maps = make_in_maps(x, Wx, Wh, b_lstm)
    trace = bool(int(os.environ.get("KERNEL_TRACE", "0")))
    res = run_bass_kernel_spmd(
        nc, in_maps, core_ids=list(range(NCORES)),
        trace=trace, trace_cores=list(range(NCORES)) if trace else None,
    )
    _CACHE["last_result"] = res
    outp = np.empty((B, W, H), np.float32)
    for core in range(NCORES):
        arr = np.asarray(res.results[core]["out"], np.float32)  # [128, C]
        # arr[hf*64+h, b8*32+t] -> out[hf*8+b8, t, h]
        outp[core * BL:(core + 1) * BL] = (
            arr.reshape(2, H, HB, W).transpose(0, 2, 3, 1)
            .reshape(BL, W, H))
    return outp


# revision 18
# speedup vs baseline: 1.5622x; 1.0541x over previous
"""Trainium2 kernel for nn_AttentionRNN_79078937853994 (v2).

The reference reduces to an LSTM over W=32 steps (attention is dead code:
softmax over a size-1 axis == 1).  K Jacobi fixed-point sweeps replace the
serial loop (contraction ~0.1/sweep; K=2 -> ~8.5e-3 rel err, K=3 -> ~9e-4).

v2 layout: partitions = (batch-half hf, h) = 128; free = (slot, b_loc, t)
with slots (i, g, f, o).  Phase 1 computes Gx directly in this layout with
col-tiled bf16 matmuls (tile_position inferred from base partitions): for
each f-chunk j and slot s, two concurrent matmuls (one per batch half) of
N=256.  Gates live in PSUM for the whole kernel; sweep k's recurrent
matmuls ACCUMULATE Wh^T @ (h_k - h_{k-1}) on top (start=False), so there
are no DVE gate adds and no SBUF gx assembly at all.  Bias and the f-gate
t=0 reset (-60, scan segment boundary) are rank-1 matmuls.

Everything is bf16 except PSUM and the final scan output (fp32); the scan
keeps fp32 state internally so bf16 operands do not compound error.
Dummy matmuls on a junk tile warm the PE HAM clock gate during the input
DMA window.  Output is ONE linear [128, 256] DMA; the host unscrambles.
"""

import json
import os
import numpy as np

import concourse.bass as bass
import concourse.mybir as mybir
import concourse.tile as tile
from concourse.bass_utils import run_bass_kernel_spmd


def _legalize_bir_waits(bir_json: bytes) -> bytes:
    """This toolchain's walrus accepts at most ONE sync wait per
    instruction.  Tile's kernel-tail drain carries one wait per live
    engine/DMA lane.  Split any excess waits onto inserted same-engine
    Drain instructions (pipeline already empty there, so they are free)."""
    d = json.loads(bir_json)
    changed = False
    for fn in d.get("functions", []):
        for bb in fn.get("blocks", []):
            insts = bb.get("instructions", [])
            out = []
            for ins in insts:
                sy = ins.get("sync_info") or {}
                ow = sy.get("on_wait") or []
                if len(ow) > 1:
                    changed = True
                    for k, w in enumerate(ow[:-1]):
                        out.append({
                            "name": f"{ins['name']}-lw{k}",
                            "opcode": "Drain",
                            "engine": ins.get("engine", "SP"),
                            "ins": [],
                            "outs": [],
                            "debug": ins.get("debug"),
                            "sync_info": {"on_wait": [w], "on_update": []},
                        })
                    sy["on_wait"] = [ow[-1]]
                out.append(ins)
            bb["instructions"] = out
    if not changed:
        return bir_json
    return json.dumps(d).encode()


def _install_bir_legalizer():
    import concourse.bass_utils as bu
    import concourse.bass2jax as b2j
    if getattr(bu, "_wait_legalizer_installed", False):
        return
    if os.environ.get("KERNEL_LDWOPT", "0") == "1":
        orig_args = bu.get_walrus_args

        def patched_args(arch, tmpdir, *, dve_root=None):
            return [a.replace("--enable-ldw-opt=false", "--enable-ldw-opt=true")
                    for a in orig_args(arch, tmpdir, dve_root=dve_root)]

        bu.get_walrus_args = patched_args
    orig = bu.compile_bir_kernel

    def patched(bir_json, tmpdir, neff_name="file.neff"):
        if isinstance(bir_json, str):
            bir_json = bir_json.encode()
        return orig(_legalize_bir_waits(bir_json), tmpdir, neff_name)

    bu.compile_bir_kernel = patched
    b2j.compile_bir_kernel = patched
    bu._wait_legalizer_installed = True


_install_bir_legalizer()

B, F, W, H = 128, 1024, 32, 64
NCORES = 8
BL = B // NCORES           # 16 batch rows per core
HB = BL // 2               # 8 rows per partition-half
C = HB * W                 # 256 free columns per half: (b_loc, t), t inner
WP = W + 2                 # h buffers padded: col 0 unused, col 1 = zero
NSWEEP = int(os.environ.get("KERNEL_NSWEEP", "2"))
NWARM = int(os.environ.get("KERNEL_NWARM", "0"))
DEBUG_GX = os.environ.get("KERNEL_DEBUG_GX", "0") == "1"
FP32 = mybir.dt.float32
BF16 = mybir.dt.bfloat16
AF = mybir.ActivationFunctionType
OP = mybir.AluOpType


def build_program():
    nc = bass.Bass()

    # x packed two f-chunks per DMA: 2 KiB per partition line keeps the
    # SDMA descriptors at full rate (1 KiB lines measured at half rate)
    xs = nc.declare_dram_parameter("xs", [4, 128, 2, BL, W], BF16,
                                   isOutput=False)
    wx = nc.declare_dram_parameter("wx", [128, 8, 4, H], BF16, isOutput=False)
    wh = nc.declare_dram_parameter("wh", [128, 4, H], BF16, isOutput=False)
    msc = nc.declare_dram_parameter("msc", [1, 1024], BF16, isOutput=False)
    out = nc.declare_dram_parameter("out", [128, C], FP32, isOutput=True)
    dbg = (nc.declare_dram_parameter("dbg", [128, 4 * C], FP32, isOutput=True)
           if DEBUG_GX else None)

    with tile.TileContext(nc) as tc:
        with (
            tc.tile_pool(name="gatesp", bufs=1, space="PSUM") as gatesp,
            tc.tile_pool(name="warmp", bufs=1, space="PSUM") as warmp,
            tc.tile_pool(name="const", bufs=1) as const,
        ):
            gates = gatesp.tile([128, 4, C], FP32)     # banks 0-1, resident
            wps = warmp.tile([128, 512], FP32)         # warm-up garbage bank

            wx_sb = const.tile([128, 8, 4, H], BF16)
            wh_sb = const.tile([128, 4, H], BF16)
            msc_sb = const.tile([1, 1024], BF16)
            junk = const.tile([128, 512], BF16)
            warm_sb = const.tile([1, 4], FP32)
            xt2 = [const.tile([128, 2, BL, W], BF16, name=f"xt{j}")
                   for j in range(4)]
            hbufs = [const.tile([128, HB, WP], BF16, name=f"hb{k}")
                     for k in range(max(NSWEEP - 1, 1))]
            dlt = (const.tile([128, HB, WP], BF16, name="dlt")
                   if NSWEEP > 2 else None)

            # zero-init h buffers (only col 1 must be zero, but a full
            # memset is cheap and runs during the DMA window) + junk tile
            nc.gpsimd.memset(junk[:].bitcast(FP32), 0.0)
            for hb in hbufs:
                nc.gpsimd.memset(hb[:].bitcast(FP32), 0.0)
            if dlt is not None:
                nc.gpsimd.memset(dlt[:].bitcast(FP32), 0.0)
            nc.gpsimd.memset(warm_sb[:], 0.5)

            # The DMA pipe here has a large fixed trigger->semaphore
            # latency, so spread the inputs across BOTH HWDGE rings
            # (SP=sync and ACT=scalar) in consumption order: each ring's
            # first transfer is needed first.
            nc.sync.dma_start(wx_sb[:, 0:4], wx[:, 0:4])
            nc.scalar.dma_start(msc_sb[:], msc[:])
            nc.sync.dma_start(xt2[0][:], xs[0])
            nc.scalar.dma_start(wx_sb[:, 4:8], wx[:, 4:8])
            nc.sync.dma_start(xt2[1][:], xs[1])
            nc.scalar.dma_start(xt2[2][:], xs[2])
            nc.sync.dma_start(xt2[3][:], xs[3])
            nc.scalar.dma_start(wh_sb[:], wh[:])

            # ACT table warm-up (sigmoid set includes tanh) during DMAs.
            nc.scalar.activation(warm_sb[0:1, 0:2], warm_sb[0:1, 0:2],
                                 AF.Sigmoid)
            nc.scalar.activation(warm_sb[0:1, 2:4], warm_sb[0:1, 0:2],
                                 AF.Tanh)

            # (optional) PE HAM warm-up — measured ineffective on these
            # parts (PE stays ~1 GHz), so NWARM defaults to 0.
            for k in range(NWARM):
                nc.tensor.matmul(wps[:], junk[:, 0:128], junk[:],
                                 start=True, stop=True,
                                 skip_group_check=True)

            # ---- Phase 1: Gx -> PSUM, direct (hf, h) layout -------------
            # One K=1 zero-matmul per bank claims the whole bank first
            # (start=True marks the full 2 KiB zero-region; writing every
            # byte clears it and sets has_written everywhere).  Every later
            # matmul uses start=False and is therefore ORDER-INDEPENDENT —
            # Tile may reorder them freely without corrupting accumulation.
            # The WAW overlap with the zero-matmul keeps them ordered after
            # it.  The zero/bias/reset matmuls depend only on junk/msc, so
            # they execute during the x DMA wait — off the critical path.
            for half in range(2):
                nc.tensor.matmul(
                    gates[:, 2 * half:2 * half + 2, :],
                    junk[0:1, 0:128], junk[0:1, 0:512],
                    start=True, stop=False, skip_group_check=True,
                )
            # bias (rank-1, misc[512+128s:...] = [b_s | b_s]) and the
            # f-gate t=0 reset: ones x (-60 pattern) into slot 2.
            for s in range(4):
                nc.tensor.matmul(
                    gates[:, s, :],
                    msc_sb[0:1, 512 + 128 * s:512 + 128 * (s + 1)],
                    msc_sb[0:1, 0:C],
                    start=False, stop=False, skip_group_check=True,
                )
            nc.tensor.matmul(
                gates[:, 2, :], msc_sb[0:1, 0:128], msc_sb[0:1, C:2 * C],
                start=False, stop=False, skip_group_check=True,
            )
            # Per f-chunk j and slot s: two col-tiled matmuls (batch halves
            # run concurrently in the PE array; tile_position inferred from
            # output base partition).
            for j in range(8):
                for s in range(4):
                    for hf in range(2):
                        nc.tensor.matmul(
                            gates[bass.ts(hf, H), s, :],
                            wx_sb[:, j, s, :],
                            xt2[j // 2][:, j % 2, bass.ts(hf, HB), :],
                            start=False,
                            stop=(j == 7 and hf == 1 and s in (1, 3)),
                            skip_group_check=True,
                        )

            if DEBUG_GX:
                dbg_sb = const.tile([128, 4, C], FP32, name="dbg_sb")
                nc.vector.tensor_copy(dbg_sb[:, 0:2, :], gates[:, 0:2, :])
                nc.vector.tensor_copy(dbg_sb[:, 2:4, :], gates[:, 2:4, :])
                nc.sync.dma_start(dbg[:], dbg_sb[:].rearrange(
                    "p s c -> p (s c)"))

            # ---- Phase 2: K fixed-point sweeps --------------------------
            c_out = const.tile([128, C], FP32, name="c_out")
            for k in range(NSWEEP):
                last = (k == NSWEEP - 1)
                if k > 0:
                    # gates += Wh^T @ delta_h  (delta = h_0 on sweep 1)
                    src = hbufs[0] if k == 1 else dlt
                    for s in range(4):
                        if last and s == 3:
                            continue     # o-gate unused on the last sweep
                        for hf in range(2):
                            nc.tensor.matmul(
                                gates[bass.ts(hf, H), s, :],
                                wh_sb[bass.ts(hf, H), s, :],
                                src[bass.ts(hf, H), :, 1:W + 1],
                                start=False, stop=True,
                                skip_group_check=True,
                            )

                s_ig = const.tile([128, 2, C], BF16, name=f"sig{k}")
                nc.scalar.activation(s_ig[:], gates[:, 0:2, :], AF.Sigmoid)
                if last:
                    s_f = const.tile([128, C], BF16, name=f"sf{k}")
                    nc.scalar.activation(s_f[:], gates[:, 2, :], AF.Sigmoid)
                    sf, so = s_f[:], None
                else:
                    s_fo = const.tile([128, 2, C], BF16, name=f"sfo{k}")
                    nc.scalar.activation(s_fo[:], gates[:, 2:4, :],
                                         AF.Sigmoid)
                    sf, so = s_fo[:, 0, :], s_fo[:, 1, :]

                si, sg = s_ig[:, 0, :], s_ig[:, 1, :]
                # u = si * tanh(g_pre) with tanh(g) = 2*sigmoid(2g) - 1:
                # tensor_scalar gets 4x bf16 mode, tensor_tensor 2x
                # (scalar_tensor_tensor measured stuck at 1x).
                v = const.tile([128, C], BF16, name=f"v{k}")
                nc.vector.tensor_scalar(v[:], sg, 2.0, -1.0,
                                        OP.mult, OP.add)
                u = const.tile([128, C], BF16, name=f"u{k}")
                nc.vector.tensor_tensor(u[:], si, v[:], OP.mult)
                if last:
                    # split the final scan by column halves so the first
                    # output DMA (and its long completion latency) starts
                    # while the second half still scans; one DMA per ring
                    for hc in range(2):
                        cs = slice(hc * (C // 2), (hc + 1) * (C // 2))
                        nc.vector.tensor_tensor_scan(
                            c_out[:, cs], s_f[:, cs], u[:, cs], 0.0,
                            OP.mult, OP.add)
                        dma_eng = nc.sync if hc == 0 else nc.scalar
                        dma_eng.dma_start(out[:, cs], c_out[:, cs])
                else:
                    ck = const.tile([128, C], BF16, name=f"c{k}")
                    nc.vector.tensor_tensor_scan(ck[:], sf, u[:], 0.0,
                                                 OP.mult, OP.add)
                    tck = const.tile([128, C], BF16, name=f"tc{k}")
                    nc.scalar.activation(tck[:], ck[:], AF.Tanh)
                    h_cur = hbufs[k]
                    so3 = so.rearrange("p (b t) -> p b t", t=W)
                    tc3 = tck[:].rearrange("p (b t) -> p b t", t=W)
                    nc.vector.tensor_tensor(h_cur[:, :, 2:W + 2], so3, tc3,
                                            OP.mult)
                    if k > 0 and not last:
                        nc.vector.tensor_tensor(
                            dlt[:, :, 2:W + 2], h_cur[:, :, 2:W + 2],
                            hbufs[k - 1][:, :, 2:W + 2], OP.subtract)

    return nc


_CACHE = {}


def _get_program():
    if "nc" not in _CACHE:
        _CACHE["nc"] = build_program()
    return _CACHE["nc"]


def _to_bf16(a):
    import ml_dtypes
    return np.ascontiguousarray(np.asarray(a, np.float32).astype(
        ml_dtypes.bfloat16))


# keras gate order in the 4H axis is (i, f, g, o); our slot order (i,g,f,o)
_SLOT = [0, 2, 1, 3]


def make_in_maps(x, Wx, Wh, b_lstm):
    x = np.asarray(x, np.float32)
    Wxs = np.asarray(Wx, np.float32).copy()
    Whs = np.asarray(Wh, np.float32).copy()
    bs = np.asarray(b_lstm, np.float32).copy()
    Wxs[:, 2 * H:3 * H] *= 2.0       # tanh(g) = 2*sigmoid(2g) - 1
    Whs[:, 2 * H:3 * H] *= 2.0
    bs[2 * H:3 * H] *= 2.0

    wx_p = _to_bf16(Wxs.reshape(F, 4, H)[:, _SLOT, :]
                    .reshape(128, 8, 4, H))
    wh4 = Whs.reshape(H, 4, H)[:, _SLOT, :]
    wh_p = _to_bf16(np.concatenate([wh4, wh4], axis=0))
    b4 = bs.reshape(4, H)[_SLOT]
    msc = np.zeros((1, 1024), np.float32)
    msc[0, 0:C] = 1.0                          # ones (bias rhs / reset lhsT)
    msc[0, C + np.arange(HB) * W] = -60.0      # t=0 reset pattern
    for s in range(4):
        msc[0, 512 + 128 * s:512 + 128 * s + H] = b4[s]
        msc[0, 512 + 128 * s + H:512 + 128 * (s + 1)] = b4[s]
    msc_p = _to_bf16(msc)

    xb = _to_bf16(x)
    in_maps = []
    for core in range(NCORES):
        shard = xb[core * BL:(core + 1) * BL]      # [16, 1024, 32]
        # xs[jj, p, i, b, t] = shard[b, 8p + 2*jj + i, t]  (2 chunks/DMA)
        xsp = np.ascontiguousarray(
            shard.reshape(BL, 128, 4, 2, W).transpose(2, 1, 3, 0, 4))
        in_maps.append({
            "xs": xsp, "wx": wx_p, "wh": wh_p, "msc": msc_p,
        })
    return in_maps


def kernel(x, W_state, b_state, W_in, w_attn, b_attn, Wx, Wh, b_lstm):
    nc = _get_program()
    in_maps = make_in_maps(x, Wx, Wh, b_lstm)
    trace = bool(int(os.environ.get("KERNEL_TRACE", "0")))
    res = run_bass_kernel_spmd(
        nc, in_maps, core_ids=list(range(NCORES)),
        trace=trace, trace_cores=list(range(NCORES)) if trace else None,
    )
    _CACHE["last_result"] = res
    outp = np.empty((B, W, H), np.float32)
    for core in range(NCORES):
        arr = np.asarray(res.results[core]["out"], np.float32)  # [128, C]
        # arr[hf*64+h, b8*32+t] -> out[hf*8+b8, t, h]
        outp[core * BL:(core + 1) * BL] = (
            arr.reshape(2, H, HB, W).transpose(0, 2, 3, 1)
            .reshape(BL, W, H))
    return outp


# revision 21
# speedup vs baseline: 1.5912x; 1.0185x over previous
"""Trainium2 kernel for nn_AttentionRNN_79078937853994 (v2).

The reference reduces to an LSTM over W=32 steps (attention is dead code:
softmax over a size-1 axis == 1).  K Jacobi fixed-point sweeps replace the
serial loop (contraction ~0.1/sweep; K=2 -> ~8.5e-3 rel err, K=3 -> ~9e-4).

v2 layout: partitions = (batch-half hf, h) = 128; free = (slot, b_loc, t)
with slots (i, g, f, o).  Phase 1 computes Gx directly in this layout with
col-tiled bf16 matmuls (tile_position inferred from base partitions): for
each f-chunk j and slot s, two concurrent matmuls (one per batch half) of
N=256.  Gates live in PSUM for the whole kernel; sweep k's recurrent
matmuls ACCUMULATE Wh^T @ (h_k - h_{k-1}) on top (start=False), so there
are no DVE gate adds and no SBUF gx assembly at all.  Bias and the f-gate
t=0 reset (-60, scan segment boundary) are rank-1 matmuls.

Everything is bf16 except PSUM and the final scan output (fp32); the scan
keeps fp32 state internally so bf16 operands do not compound error.
Dummy matmuls on a junk tile warm the PE HAM clock gate during the input
DMA window.  Output is ONE linear [128, 256] DMA; the host unscrambles.
"""

import json
import os
import numpy as np

import concourse.bass as bass
import concourse.mybir as mybir
import concourse.tile as tile
from concourse.bass_utils import run_bass_kernel_spmd


def _legalize_bir_waits(bir_json: bytes) -> bytes:
    """This toolchain's walrus accepts at most ONE sync wait per
    instruction.  Tile's kernel-tail drain carries one wait per live
    engine/DMA lane.  Split any excess waits onto inserted same-engine
    Drain instructions (pipeline already empty there, so they are free)."""
    d = json.loads(bir_json)
    changed = False
    for fn in d.get("functions", []):
        for bb in fn.get("blocks", []):
            insts = bb.get("instructions", [])
            out = []
            for ins in insts:
                sy = ins.get("sync_info") or {}
                ow = sy.get("on_wait") or []
                if len(ow) > 1:
                    changed = True
                    for k, w in enumerate(ow[:-1]):
                        out.append({
                            "name": f"{ins['name']}-lw{k}",
                            "opcode": "Drain",
                            "engine": ins.get("engine", "SP"),
                            "ins": [],
                            "outs": [],
                            "debug": ins.get("debug"),
                            "sync_info": {"on_wait": [w], "on_update": []},
                        })
                    sy["on_wait"] = [ow[-1]]
                out.append(ins)
            bb["instructions"] = out
    if not changed:
        return bir_json
    return json.dumps(d).encode()


def _install_bir_legalizer():
    import concourse.bass_utils as bu
    import concourse.bass2jax as b2j
    if getattr(bu, "_wait_legalizer_installed", False):
        return
    if os.environ.get("KERNEL_LDWOPT", "0") == "1":
        orig_args = bu.get_walrus_args

        def patched_args(arch, tmpdir, *, dve_root=None):
            return [a.replace("--enable-ldw-opt=false", "--enable-ldw-opt=true")
                    for a in orig_args(arch, tmpdir, dve_root=dve_root)]

        bu.get_walrus_args = patched_args
    orig = bu.compile_bir_kernel

    def patched(bir_json, tmpdir, neff_name="file.neff"):
        if isinstance(bir_json, str):
            bir_json = bir_json.encode()
        return orig(_legalize_bir_waits(bir_json), tmpdir, neff_name)

    bu.compile_bir_kernel = patched
    b2j.compile_bir_kernel = patched
    bu._wait_legalizer_installed = True


_install_bir_legalizer()

B, F, W, H = 128, 1024, 32, 64
NCORES = 8
BL = B // NCORES           # 16 batch rows per core
HB = BL // 2               # 8 rows per partition-half
C = HB * W                 # 256 free columns per half: (b_loc, t), t inner
WP = W + 2                 # h buffers padded: col 0 unused, col 1 = zero
NSWEEP = int(os.environ.get("KERNEL_NSWEEP", "2"))
NWARM = int(os.environ.get("KERNEL_NWARM", "0"))
DEBUG_GX = os.environ.get("KERNEL_DEBUG_GX", "0") == "1"
FP32 = mybir.dt.float32
BF16 = mybir.dt.bfloat16
AF = mybir.ActivationFunctionType
OP = mybir.AluOpType


def build_program():
    nc = bass.Bass()

    # x packed two f-chunks per DMA: 2 KiB per partition line keeps the
    # SDMA descriptors at full rate (1 KiB lines measured at half rate)
    xs = nc.declare_dram_parameter("xs", [4, 128, 2, BL, W], BF16,
                                   isOutput=False)
    wx = nc.declare_dram_parameter("wx", [128, 8, 4, H], BF16, isOutput=False)
    wh = nc.declare_dram_parameter("wh", [128, 4, H], BF16, isOutput=False)
    msc = nc.declare_dram_parameter("msc", [1, 1024], BF16, isOutput=False)
    out = nc.declare_dram_parameter("out", [128, C], FP32, isOutput=True)
    dbg = (nc.declare_dram_parameter("dbg", [128, 4 * C], FP32, isOutput=True)
           if DEBUG_GX else None)

    with tile.TileContext(nc) as tc:
        with (
            tc.tile_pool(name="gatesp", bufs=1, space="PSUM") as gatesp,
            tc.tile_pool(name="warmp", bufs=1, space="PSUM") as warmp,
            tc.tile_pool(name="const", bufs=1) as const,
        ):
            gates = gatesp.tile([128, 4, C], FP32)     # banks 0-1, resident
            wps = warmp.tile([128, 512], FP32)         # warm-up garbage bank

            wx_sb = const.tile([128, 8, 4, H], BF16)
            wh_sb = const.tile([128, 4, H], BF16)
            msc_sb = const.tile([1, 1024], BF16)
            junk = const.tile([128, 512], BF16)
            warm_sb = const.tile([1, 4], FP32)
            xt2 = [const.tile([128, 2, BL, W], BF16, name=f"xt{j}")
                   for j in range(4)]
            hbufs = [const.tile([128, HB, WP], BF16, name=f"hb{k}")
                     for k in range(max(NSWEEP - 1, 1))]
            dlt = (const.tile([128, HB, WP], BF16, name="dlt")
                   if NSWEEP > 2 else None)

            # zero-init h buffers (only col 1 must be zero, but a full
            # memset is cheap and runs during the DMA window) + junk tile
            nc.gpsimd.memset(junk[:].bitcast(FP32), 0.0)
            for hb in hbufs:
                nc.gpsimd.memset(hb[:].bitcast(FP32), 0.0)
            if dlt is not None:
                nc.gpsimd.memset(dlt[:].bitcast(FP32), 0.0)
            nc.gpsimd.memset(warm_sb[:], 0.5)

            # The DMA pipe here has a large fixed trigger->semaphore
            # latency, so spread the inputs across BOTH HWDGE rings
            # (SP=sync and ACT=scalar) in consumption order: each ring's
            # first transfer is needed first.
            # (x tile jj is needed at pair-slot 8*jj of 32, ~213ns each;
            # each ring serializes its transfers, so alternate tiles
            # across rings in consumption order)
            nc.sync.dma_start(wx_sb[:, 0:4], wx[:, 0:4])
            nc.scalar.dma_start(msc_sb[:], msc[:])
            nc.sync.dma_start(xt2[0][:], xs[0])
            nc.scalar.dma_start(xt2[1][:], xs[1])
            nc.sync.dma_start(xt2[2][:], xs[2])
            nc.scalar.dma_start(wx_sb[:, 4:8], wx[:, 4:8])
            nc.sync.dma_start(xt2[3][:], xs[3])
            nc.scalar.dma_start(wh_sb[:], wh[:])

            # ACT table warm-up (sigmoid set includes tanh) during DMAs.
            nc.scalar.activation(warm_sb[0:1, 0:2], warm_sb[0:1, 0:2],
                                 AF.Sigmoid)
            nc.scalar.activation(warm_sb[0:1, 2:4], warm_sb[0:1, 0:2],
                                 AF.Tanh)

            # (optional) PE HAM warm-up — measured ineffective on these
            # parts (PE stays ~1 GHz), so NWARM defaults to 0.
            for k in range(NWARM):
                nc.tensor.matmul(wps[:], junk[:, 0:128], junk[:],
                                 start=True, stop=True,
                                 skip_group_check=True)

            # ---- Phase 1: Gx -> PSUM, direct (hf, h) layout -------------
            # One K=1 zero-matmul per bank claims the whole bank first
            # (start=True marks the full 2 KiB zero-region; writing every
            # byte clears it and sets has_written everywhere).  Every later
            # matmul uses start=False and is therefore ORDER-INDEPENDENT —
            # Tile may reorder them freely without corrupting accumulation.
            # The WAW overlap with the zero-matmul keeps them ordered after
            # it.  The zero/bias/reset matmuls depend only on junk/msc, so
            # they execute during the x DMA wait — off the critical path.
            for half in range(2):
                nc.tensor.matmul(
                    gates[:, 2 * half:2 * half + 2, :],
                    junk[0:1, 0:128], junk[0:1, 0:512],
                    start=True, stop=False, skip_group_check=True,
                )
            # bias (rank-1, misc[512+128s:...] = [b_s | b_s]) and the
            # f-gate t=0 reset: ones x (-60 pattern) into slot 2.
            for s in range(4):
                nc.tensor.matmul(
                    gates[:, s, :],
                    msc_sb[0:1, 512 + 128 * s:512 + 128 * (s + 1)],
                    msc_sb[0:1, 0:C],
                    start=False, stop=False, skip_group_check=True,
                )
            nc.tensor.matmul(
                gates[:, 2, :], msc_sb[0:1, 0:128], msc_sb[0:1, C:2 * C],
                start=False, stop=False, skip_group_check=True,
            )
            # Per f-chunk j and slot s: two col-tiled matmuls (batch halves
            # run concurrently in the PE array; tile_position inferred from
            # output base partition).
            for j in range(8):
                for s in range(4):
                    for hf in range(2):
                        nc.tensor.matmul(
                            gates[bass.ts(hf, H), s, :],
                            wx_sb[:, j, s, :],
                            xt2[j // 2][:, j % 2, bass.ts(hf, HB), :],
                            start=False,
                            stop=(j == 7 and hf == 1 and s in (1, 3)),
                            skip_group_check=True,
                        )

            if DEBUG_GX:
                dbg_sb = const.tile([128, 4, C], FP32, name="dbg_sb")
                nc.vector.tensor_copy(dbg_sb[:, 0:2, :], gates[:, 0:2, :])
                nc.vector.tensor_copy(dbg_sb[:, 2:4, :], gates[:, 2:4, :])
                nc.sync.dma_start(dbg[:], dbg_sb[:].rearrange(
                    "p s c -> p (s c)"))

            # ---- Phase 2: K fixed-point sweeps --------------------------
            c_out = const.tile([128, C], FP32, name="c_out")
            for k in range(NSWEEP):
                last = (k == NSWEEP - 1)
                if k > 0:
                    # gates += Wh^T @ delta_h  (delta = h_0 on sweep 1)
                    src = hbufs[0] if k == 1 else dlt
                    for s in range(4):
                        if last and s == 3:
                            continue     # o-gate unused on the last sweep
                        for hf in range(2):
                            nc.tensor.matmul(
                                gates[bass.ts(hf, H), s, :],
                                wh_sb[bass.ts(hf, H), s, :],
                                src[bass.ts(hf, H), :, 1:W + 1],
                                start=False, stop=True,
                                skip_group_check=True,
                            )

                # sigmoid split (i,g) / f / o: the scan needs sf as early
                # as possible, the o-gate only after tanh(c)
                s_ig = const.tile([128, 2, C], BF16, name=f"sig{k}")
                nc.scalar.activation(s_ig[:], gates[:, 0:2, :], AF.Sigmoid)
                s_f = const.tile([128, C], BF16, name=f"sf{k}")
                nc.scalar.activation(s_f[:], gates[:, 2, :], AF.Sigmoid)
                sf = s_f[:]
                if not last:
                    s_o = const.tile([128, C], BF16, name=f"so{k}")
                    nc.scalar.activation(s_o[:], gates[:, 3, :], AF.Sigmoid)
                    so = s_o[:]

                si, sg = s_ig[:, 0, :], s_ig[:, 1, :]
                # u = si * tanh(g_pre) with tanh(g) = 2*sigmoid(2g) - 1:
                # tensor_scalar gets 4x bf16 mode, tensor_tensor 2x
                # (scalar_tensor_tensor measured stuck at 1x).
                v = const.tile([128, C], BF16, name=f"v{k}")
                nc.vector.tensor_scalar(v[:], sg, 2.0, -1.0,
                                        OP.mult, OP.add)
                u = const.tile([128, C], BF16, name=f"u{k}")
                nc.vector.tensor_tensor(u[:], si, v[:], OP.mult)
                if last:
                    # split the final scan by column halves so the first
                    # output DMA (and its long completion latency) starts
                    # while the second half still scans; one DMA per ring
                    for hc in range(2):
                        cs = slice(hc * (C // 2), (hc + 1) * (C // 2))
                        nc.vector.tensor_tensor_scan(
                            c_out[:, cs], s_f[:, cs], u[:, cs], 0.0,
                            OP.mult, OP.add)
                        # hc=0 via SWDGE as a completion-latency probe
                        dma_eng = nc.gpsimd if hc == 0 else nc.scalar
                        dma_eng.dma_start(out[:, cs], c_out[:, cs])
                else:
                    ck = const.tile([128, C], BF16, name=f"c{k}")
                    nc.vector.tensor_tensor_scan(ck[:], sf, u[:], 0.0,
                                                 OP.mult, OP.add)
                    tck = const.tile([128, C], BF16, name=f"tc{k}")
                    nc.scalar.activation(tck[:], ck[:], AF.Tanh)
                    h_cur = hbufs[k]
                    so3 = so.rearrange("p (b t) -> p b t", t=W)
                    tc3 = tck[:].rearrange("p (b t) -> p b t", t=W)
                    nc.vector.tensor_tensor(h_cur[:, :, 2:W + 2], so3, tc3,
                                            OP.mult)
                    if k > 0 and not last:
                        nc.vector.tensor_tensor(
                            dlt[:, :, 2:W + 2], h_cur[:, :, 2:W + 2],
                            hbufs[k - 1][:, :, 2:W + 2], OP.subtract)

    return nc


_CACHE = {}


def _get_program():
    if "nc" not in _CACHE:
        _CACHE["nc"] = build_program()
    return _CACHE["nc"]


def _to_bf16(a):
    import ml_dtypes
    return np.ascontiguousarray(np.asarray(a, np.float32).astype(
        ml_dtypes.bfloat16))


# keras gate order in the 4H axis is (i, f, g, o); our slot order (i,g,f,o)
_SLOT = [0, 2, 1, 3]


def make_in_maps(x, Wx, Wh, b_lstm):
    x = np.asarray(x, np.float32)
    Wxs = np.asarray(Wx, np.float32).copy()
    Whs = np.asarray(Wh, np.float32).copy()
    bs = np.asarray(b_lstm, np.float32).copy()
    Wxs[:, 2 * H:3 * H] *= 2.0       # tanh(g) = 2*sigmoid(2g) - 1
    Whs[:, 2 * H:3 * H] *= 2.0
    bs[2 * H:3 * H] *= 2.0

    wx_p = _to_bf16(Wxs.reshape(F, 4, H)[:, _SLOT, :]
                    .reshape(128, 8, 4, H))
    wh4 = Whs.reshape(H, 4, H)[:, _SLOT, :]
    wh_p = _to_bf16(np.concatenate([wh4, wh4], axis=0))
    b4 = bs.reshape(4, H)[_SLOT]
    msc = np.zeros((1, 1024), np.float32)
    msc[0, 0:C] = 1.0                          # ones (bias rhs / reset lhsT)
    msc[0, C + np.arange(HB) * W] = -60.0      # t=0 reset pattern
    for s in range(4):
        msc[0, 512 + 128 * s:512 + 128 * s + H] = b4[s]
        msc[0, 512 + 128 * s + H:512 + 128 * (s + 1)] = b4[s]
    msc_p = _to_bf16(msc)

    xb = _to_bf16(x)
    in_maps = []
    for core in range(NCORES):
        shard = xb[core * BL:(core + 1) * BL]      # [16, 1024, 32]
        # xs[jj, p, i, b, t] = shard[b, 8p + 2*jj + i, t]  (2 chunks/DMA)
        xsp = np.ascontiguousarray(
            shard.reshape(BL, 128, 4, 2, W).transpose(2, 1, 3, 0, 4))
        in_maps.append({
            "xs": xsp, "wx": wx_p, "wh": wh_p, "msc": msc_p,
        })
    return in_maps


def kernel(x, W_state, b_state, W_in, w_attn, b_attn, Wx, Wh, b_lstm):
    nc = _get_program()
    in_maps = make_in_maps(x, Wx, Wh, b_lstm)
    trace = bool(int(os.environ.get("KERNEL_TRACE", "0")))
    res = run_bass_kernel_spmd(
        nc, in_maps, core_ids=list(range(NCORES)),
        trace=trace, trace_cores=list(range(NCORES)) if trace else None,
    )
    _CACHE["last_result"] = res
    outp = np.empty((B, W, H), np.float32)
    for core in range(NCORES):
        arr = np.asarray(res.results[core]["out"], np.float32)  # [128, C]
        # arr[hf*64+h, b8*32+t] -> out[hf*8+b8, t, h]
        outp[core * BL:(core + 1) * BL] = (
            arr.reshape(2, H, HB, W).transpose(0, 2, 3, 1)
            .reshape(BL, W, H))
    return outp
